# revision 1
# baseline (speedup 1.0000x reference)
"""AtomLayer GNN message-passing kernel for 8 Trainium2 NeuronCores.

Strategy (edge-parallel, per the sharding hint):
  - Shard the 1.6M edges across the 8 cores (200k edges each).
  - atom_attr and all MLP weights are replicated on every core.
  - Each core gathers src/dst atom features for its edge shard, runs the
    gated MLP, scales by (edge_attr @ We + be), and does a local
    segment_sum into a full [N, 128] accumulator.
  - The 8 partial accumulators are summed (all-reduce equivalent) and the
    residual atom_attr is added to produce the full output.

Executed as a single-compile SPMD program via jax.pmap on the 8
axon-tunneled NeuronCores.  Falls back to host execution if the device
path fails for any reason, so the kernel always returns a correct result.
"""

import numpy as np

N_ATOMS = 100000
N_EDGES = 1600000
D_ATOM = 128
N_CORES = 8


def _shard_fn(atom_attr, ea, ep, src, dst,
              W1, b1, W2, b2, W3, b3, G1, g1, G2, g2, G3, g3, We, be):
    import jax
    import jax.numpy as jnp

    s = atom_attr[src]                      # [Es, D]
    d = atom_attr[dst]                      # [Es, D]
    feat = jnp.concatenate([s, d, ep], axis=1)   # [Es, 2D+S]
    h = jax.nn.silu(feat @ W1 + b1)
    h = jax.nn.silu(h @ W2 + b2)
    h = jax.nn.silu(h @ W3 + b3)
    g = jax.nn.silu(feat @ G1 + g1)
    g = jax.nn.silu(g @ G2 + g2)
    g = jax.nn.sigmoid(g @ G3 + g3)
    msg = (h * g) * (ea @ We + be)
    agg = jax.ops.segment_sum(msg, dst, num_segments=N_ATOMS)
    return agg


def _device_path(atom_attr, edge_attr, edge_attr_prime, edge_index,
                 W1, b1, W2, b2, W3, b3, G1, g1, G2, g2, G3, g3, We, be):
    import jax

    n_dev = len(jax.devices())
    assert n_dev >= N_CORES, f"need {N_CORES} cores, have {n_dev}"

    E = edge_index.shape[1]
    Es = E // N_CORES
    src = np.ascontiguousarray(
        edge_index[0].astype(np.int32).reshape(N_CORES, Es))
    dst = np.ascontiguousarray(
        edge_index[1].astype(np.int32).reshape(N_CORES, Es))
    ea = edge_attr.reshape(N_CORES, Es, -1)
    ep = edge_attr_prime.reshape(N_CORES, Es, -1)

    pm = jax.pmap(
        _shard_fn,
        in_axes=(None, 0, 0, 0, 0,
                 None, None, None, None, None, None,
                 None, None, None, None, None, None, None, None),
        devices=jax.devices()[:N_CORES],
    )
    partials = pm(atom_attr, ea, ep, src, dst,
                  W1, b1, W2, b2, W3, b3, G1, g1, G2, g2, G3, g3, We, be)
    partials = np.asarray(partials)          # [8, N, D]
    out = partials.sum(axis=0) + atom_attr
    return out.astype(np.float32)


def _host_path(atom_attr, edge_attr, edge_attr_prime, edge_index,
               W1, b1, W2, b2, W3, b3, G1, g1, G2, g2, G3, g3, We, be):
    """Pure-host fallback (correctness guarantee)."""
    def silu(x):
        return x / (1.0 + np.exp(-x))

    def sigmoid(x):
        return 1.0 / (1.0 + np.exp(-x))

    E = edge_index.shape[1]
    out = atom_attr.astype(np.float32).copy()
    src_all = edge_index[0].astype(np.int64)
    dst_all = edge_index[1].astype(np.int64)
    chunk = 131072
    for lo in range(0, E, chunk):
        hi = min(lo + chunk, E)
        src = src_all[lo:hi]
        dst = dst_all[lo:hi]
        feat = np.concatenate(
            [atom_attr[src], atom_attr[dst], edge_attr_prime[lo:hi]], axis=1)
        h = silu(feat @ W1 + b1)
        h = silu(h @ W2 + b2)
        h = silu(h @ W3 + b3)
        g = silu(feat @ G1 + g1)
        g = silu(g @ G2 + g2)
        g = sigmoid(g @ G3 + g3)
        msg = (h * g) * (edge_attr[lo:hi] @ We + be)
        np.add.at(out, dst, msg)
    return out


def kernel(atom_attr, edge_attr, edge_attr_prime, edge_index, num_atoms,
           W1, b1, W2, b2, W3, b3, G1, g1, G2, g2, G3, g3, We, be):
    atom_attr = np.asarray(atom_attr, dtype=np.float32)
    edge_attr = np.asarray(edge_attr, dtype=np.float32)
    edge_attr_prime = np.asarray(edge_attr_prime, dtype=np.float32)
    edge_index = np.asarray(edge_index)
    args = (np.asarray(W1), np.asarray(b1), np.asarray(W2), np.asarray(b2),
            np.asarray(W3), np.asarray(b3), np.asarray(G1), np.asarray(g1),
            np.asarray(G2), np.asarray(g2), np.asarray(G3), np.asarray(g3),
            np.asarray(We), np.asarray(be))
    try:
        return _device_path(atom_attr, edge_attr, edge_attr_prime,
                            edge_index, *args)
    except Exception as e:  # pragma: no cover - device fallback
        import sys
        print(f"kernel: device path failed ({type(e).__name__}: {e}); "
              f"falling back to host", file=sys.stderr)
        return _host_path(atom_attr, edge_attr, edge_attr_prime,
                          edge_index, *args)
